# revision 2
# baseline (speedup 1.0000x reference)
"""Self-contained Trainium2 (Bass) kernel for nn_GCNClassifier: 3-layer GCN +
l2norm/relu + mean-pool readout, sharded across 8 NeuronCores.

Strategy (per core, nodes sharded 6250/core, stored in degree-ranked order):
  - GCN symmetric norm factorized into per-node scales:
        out = Dinv * (A_hat (Dinv * (h @ W))) + b
    so messages are raw gathered rows of the "table" t~ = Dinv*(h@W).
  - Per layer: transform own shard on PE, AllGather t~ into a full table in
    local HBM, then aggregate with dense degree-ranked gather "rounds":
    round r fetches the r-th in-edge source row for every target (targets
    sorted by in-degree so each round is a dense prefix); a DVE add
    accumulates.  No scatter-add needed anywhere in the layers.
  - dma_gather indices are int16, so the 50176-row table is split in two
    halves (source cores 0-3 / 4-7); half-B accumulates in its own degree
    order and is realigned with one extra dense permutation gather.
  - Readout: per-graph mean via one-hot matmul (weights 1/count), AllReduce,
    tiny head matmul + softmax.
"""

import numpy as np
import concourse.bacc as bacc
import concourse.mybir as mybir
import concourse.tile as tile
from concourse.tile_rust import add_dep_helper
from concourse.library_config import mlp as mlp_lib
from concourse.bass_utils import run_bass_kernel_spmd

F32 = mybir.dt.float32
I16 = mybir.dt.int16
AF = mybir.ActivationFunctionType
ALU = mybir.AluOpType
AX = mybir.AxisListType

MAX_GATHER = 3968          # idx/instruction; needs single_packet=False
N_CORES = 8


def _round_up(x, m):
    return (x + m - 1) // m * m


class _Prep:
    """Host-side graph preprocessing: indices, permutations, schedules."""

    def __init__(self, edge_index, batch, n_nodes, n_graphs, n_cores):
        N, C, G = n_nodes, n_cores, n_graphs
        assert N % C == 0
        NPC = N // C
        P = _round_up(NPC + 1, 128)   # >= 1 all-zero pad row for padding gathers
        self.N, self.C, self.G, self.NPC, self.P = N, C, G, NPC, P
        self.TAB = C * P
        self.HALF = (C // 2) * P

        ei = np.asarray(edge_index)
        row = np.concatenate([ei[0], np.arange(N, dtype=np.int64)])
        col = np.concatenate([ei[1], np.arange(N, dtype=np.int64)])
        deg = np.bincount(col, minlength=N).astype(np.float64)
        self.dinv = np.where(deg > 0, 1.0 / np.sqrt(deg), 0.0).astype(np.float32)

        scor = row // NPC
        srcA = scor < (C // 2)
        degA = np.bincount(col[srcA], minlength=N)
        degB = np.bincount(col[~srcA], minlength=N)

        self.rankA_node = np.full((C, P), -1, np.int64)
        self.posA = np.empty(N, np.int64)
        self.posB = np.empty(N, np.int64)
        for k in range(C):
            g0 = k * NPC
            oa = np.argsort(-degA[g0:g0 + NPC], kind="stable")
            ob = np.argsort(-degB[g0:g0 + NPC], kind="stable")
            self.rankA_node[k, :NPC] = g0 + oa
            self.posA[g0 + oa] = np.arange(NPC)
            self.posB[g0 + ob] = np.arange(NPC)

        self.pid = (np.arange(N) // NPC) * P + self.posA

        tcore = col // NPC
        src_pid = self.pid[row]
        half = (src_pid >= self.HALF).astype(np.int64)
        src_idx = src_pid - half * self.HALF
        assert src_idx.max() < 32768, "int16 index overflow"

        tpos = np.where(half == 0, self.posA[col], self.posB[col])
        key = (tcore * 2 + half) * P + tpos
        order = np.argsort(key, kind="stable")
        e_s, e_c, e_h, e_t = src_idx[order], tcore[order], half[order], tpos[order]
        key_s = key[order]
        _, first_idx, counts = np.unique(key_s, return_index=True, return_counts=True)
        r_of = np.arange(len(key_s)) - np.repeat(first_idx, counts)

        degH = np.zeros((C, 2, P), np.int64)
        np.add.at(degH, (e_c, e_h, e_t), 1)
        maxdeg = int(degH.max())

        n_r = np.zeros((C, 2, maxdeg), np.int64)
        for k in range(C):
            for h in (0, 1):
                hist = np.bincount(degH[k, h], minlength=maxdeg + 1)
                ge = hist[::-1].cumsum()[::-1]
                n_r[k, h] = ge[1:maxdeg + 1]
        n_common = n_r.max(axis=(0, 1))
        n_pad = ((n_common + 127) // 128 * 128).astype(np.int64)

        offs = np.concatenate([[0], np.cumsum(n_pad)])
        self.rounds = [(int(offs[i]), int(n_pad[i])) for i in range(maxdeg)]
        self.tok_half = int(offs[-1])
        self.TOK = 2 * self.tok_half

        zrow = np.int16(NPC)   # a guaranteed all-zero table row, both halves
        tok = np.full((C, self.TOK), zrow, np.int16)
        flat_pos = e_h * self.tok_half + offs[r_of] + e_t
        tok[e_c, flat_pos] = e_s.astype(np.int16)
        self.tok = tok

        nblk = P // 128
        permB = np.zeros((C, P), np.int16)
        for k in range(C):
            nodes = self.rankA_node[k]
            real = nodes >= 0
            j = np.arange(P)
            j[real] = self.posB[nodes[real]]
            permB[k] = ((j % 128) * nblk + (j // 128)).astype(np.int16)
        self.permB = permB

        batch = np.asarray(batch)
        cnt = np.bincount(batch, minlength=G).astype(np.float32)
        self.inv_cnt = (1.0 / np.maximum(cnt, 1.0)).astype(np.float32)
        self.batch = batch

    def wrap16(self, arr_c):
        C, n = arr_c.shape
        a = arr_c.reshape(C, n // 16, 16).transpose(0, 2, 1)
        return np.tile(a, (1, 8, 1)).copy()

    def core_inputs(self, x, W1, b1, W2, b2, W3, b3, Wm, bm):
        C, P, G = self.C, self.P, self.G
        nblk = P // 128
        x = np.asarray(x, np.float32)
        H = np.asarray(W1).shape[1]
        tokw = self.wrap16(self.tok)
        permw = self.wrap16(self.permB)
        bvec = np.concatenate([np.asarray(b1), np.asarray(b2),
                               np.asarray(b3)]).astype(np.float32)
        b_rep = np.broadcast_to(bvec, (128, 3 * H)).copy()
        bm_arr = np.asarray(bm, np.float32)
        bm_rep = np.broadcast_to(bm_arr, (128, bm_arr.shape[0])).copy()
        ins = []
        for k in range(C):
            nodes = self.rankA_node[k]
            real = nodes >= 0
            xk = np.zeros((P, x.shape[1]), np.float32)
            xk[real] = x[nodes[real]]
            dv = np.zeros(P, np.float32)
            dv[real] = self.dinv[nodes[real]]
            pp = np.zeros((P, G), np.float32)
            pp[real, self.batch[nodes[real]]] = self.inv_cnt[self.batch[nodes[real]]]
            ins.append({
                "xT": np.ascontiguousarray(xk.T),
                "dinvq": np.ascontiguousarray(dv.reshape(nblk, 128).T),
                "idxs": tokw[k],
                "permB": permw[k],
                "Ppool": np.ascontiguousarray(pp.reshape(nblk, 128, G).transpose(1, 0, 2)),
                "W1": np.asarray(W1, np.float32), "W2": np.asarray(W2, np.float32),
                "W3": np.asarray(W3, np.float32), "Wm": np.asarray(Wm, np.float32),
                "bvec": b_rep, "bm_rep": bm_rep,
                "I128": np.eye(128, dtype=np.float32),
            })
        return ins

    def assemble_node_emb(self, shards):
        H = shards[0].shape[1]
        out = np.empty((self.N, H), np.float32)
        for k in range(self.C):
            nodes = self.rankA_node[k]
            real = nodes >= 0
            out[nodes[real]] = np.asarray(shards[k])[real]
        return out


def _make_chunks(rounds, slab_tok):
    """Split the per-half round list into gather chunks; pieces carry the
    slab-relative / acc-absolute block offsets for the accumulate adds."""
    chunks = []
    cur = None
    for (off, n) in rounds:
        done = 0
        while done < n:
            if cur is None:
                cur = [off + done, 0, []]
            take = min(n - done, slab_tok - cur[1])
            cur[2].append((cur[1] // 128, done // 128, take // 128))
            cur[1] += take
            done += take
            if cur[1] == slab_tok:
                chunks.append(tuple(cur))
                cur = None
    if cur is not None:
        chunks.append(tuple(cur))
    return chunks


def _build_kernel(prep, F_IN, H, N_CLASSES, slab_tok=MAX_GATHER):
    C, P, G, TAB, HALF = prep.C, prep.P, prep.G, prep.TAB, prep.HALF
    nblk = P // 128
    TOK, tok_half = prep.TOK, prep.tok_half
    slab_blk = slab_tok // 128
    chunks = _make_chunks(prep.rounds, slab_tok)
    assert G % 128 == 0

    nc = bacc.Bacc("TRN2", target_bir_lowering=False, debug=False, num_devices=C)
    xT = nc.dram_tensor("xT", [F_IN, P], F32, kind="ExternalInput")
    dinvq_d = nc.dram_tensor("dinvq", [128, nblk], F32, kind="ExternalInput")
    idxs_d = nc.dram_tensor("idxs", [128, TOK // 16], I16, kind="ExternalInput")
    permB_d = nc.dram_tensor("permB", [128, P // 16], I16, kind="ExternalInput")
    Ppool_d = nc.dram_tensor("Ppool", [128, nblk, G], F32, kind="ExternalInput")
    W1_d = nc.dram_tensor("W1", [F_IN, H], F32, kind="ExternalInput")
    W2_d = nc.dram_tensor("W2", [H, H], F32, kind="ExternalInput")
    W3_d = nc.dram_tensor("W3", [H, H], F32, kind="ExternalInput")
    Wm_d = nc.dram_tensor("Wm", [H, N_CLASSES], F32, kind="ExternalInput")
    bvec_d = nc.dram_tensor("bvec", [128, 3 * H], F32, kind="ExternalInput")
    bm_d = nc.dram_tensor("bm_rep", [128, N_CLASSES], F32, kind="ExternalInput")
    I_d = nc.dram_tensor("I128", [128, 128], F32, kind="ExternalInput")

    node_emb_o = nc.dram_tensor("node_emb", [P, H], F32, kind="ExternalOutput")
    logits_o = nc.dram_tensor("logits", [G, N_CLASSES], F32, kind="ExternalOutput")
    probs_o = nc.dram_tensor("probs", [G, N_CLASSES], F32, kind="ExternalOutput")
    gemb_o = nc.dram_tensor("graph_emb", [G, H], F32, kind="ExternalOutput")

    with tile.TileContext(nc) as tc:
        with (
            tc.tile_pool(name="const", bufs=1) as cpool,
            tc.tile_pool(name="state", bufs=1) as hpool,
            tc.tile_pool(name="slabs", bufs=3) as spool,
            tc.tile_pool(name="work", bufs=3) as wpool,
            tc.tile_pool(name="psum", bufs=2, space="PSUM") as ppool,
            tc.tile_pool(name="psacc", bufs=1, space="PSUM") as papool,
            tc.tile_pool(name="dram", bufs=1, space="DRAM") as dpool,
        ):
            lib = nc.gpsimd.load_library(mlp_lib)

            def dep_lib(ins_obj):
                add_dep_helper(ins_obj.ins, lib.ins, sync=False, reason="lib first")

            W1s = cpool.tile([F_IN, H], F32); nc.sync.dma_start(W1s[:], W1_d[:])
            W2s = cpool.tile([H, H], F32);   nc.sync.dma_start(W2s[:], W2_d[:])
            W3s = cpool.tile([H, H], F32);   nc.sync.dma_start(W3s[:], W3_d[:])
            Wms = cpool.tile([H, N_CLASSES], F32); nc.sync.dma_start(Wms[:], Wm_d[:])
            bvs = cpool.tile([128, 3 * H], F32); nc.sync.dma_start(bvs[:], bvec_d[:])
            bms = cpool.tile([128, N_CLASSES], F32); nc.sync.dma_start(bms[:], bm_d[:])
            Is = cpool.tile([128, 128], F32); nc.sync.dma_start(Is[:], I_d[:])
            dinvs = cpool.tile([128, nblk], F32); nc.sync.dma_start(dinvs[:], dinvq_d[:])
            eps_t = cpool.tile([128, 1], F32); nc.vector.memset(eps_t[:], 1e-24)
            idx_sb = cpool.tile([128, TOK // 16], I16); nc.sync.dma_start(idx_sb[:], idxs_d[:])
            perm_sb = cpool.tile([128, P // 16], I16); nc.sync.dma_start(perm_sb[:], permB_d[:])
            Pps = cpool.tile([128, nblk, G], F32); nc.sync.dma_start(Pps[:], Ppool_d[:])

            h = hpool.tile([128, nblk, H], F32, tag="h")
            acc = hpool.tile([128, nblk, H], F32, tag="accA")
            accB = hpool.tile([128, nblk, H], F32, tag="accB")

            tshard = dpool.tile([P, H], F32)
            tables = [dpool.tile([TAB, H], F32, addr_space="Shared", name=f"table{i}")
                      for i in range(3)]
            accB_dram = dpool.tile([128 * nblk, H], F32)
            pool_in = dpool.tile([G, H], F32)
            pool_out = dpool.tile([G, H], F32, addr_space="Shared")

            tshard_v = tshard[:].rearrange("(b p) e -> b p e", p=128)

            Ws = [W1s, W2s, W3s]
            for layer in range(3):
                for j in range(nblk):
                    if layer == 0:
                        lhsT = wpool.tile([F_IN, 128], F32, tag="xt")
                        nc.sync.dma_start(lhsT[:], xT[:, j * 128:(j + 1) * 128])
                    else:
                        pst = ppool.tile([H, 128], F32, tag="psT")
                        nc.tensor.transpose(pst[:], h[:, j, :], Is[:])
                        lhsT = wpool.tile([H, 128], F32, tag="hT")
                        nc.scalar.copy(lhsT[:], pst[:])
                    psm = ppool.tile([128, H], F32, tag="psm")
                    nc.tensor.matmul(psm[:], lhsT[:], Ws[layer][:], start=True, stop=True)
                    ts = wpool.tile([128, H], F32, tag="ts")
                    nc.vector.tensor_scalar_mul(ts[:], psm[:], dinvs[:, j:j + 1])
                    nc.sync.dma_start(tshard_v[j], ts[:])

                table = tables[layer]
                nc.gpsimd.collective_compute(
                    "AllGather", ALU.bypass, replica_groups=[list(range(C))],
                    ins=[tshard.opt()], outs=[table.opt()])

                nc.vector.memset(acc[:], 0.0)
                nc.vector.memset(accB[:], 0.0)
                for hf in (0, 1):
                    acc_t = acc if hf == 0 else accB
                    tab_ap = table[hf * HALF:(hf + 1) * HALF, :]
                    for (toff, ntok, pieces) in chunks:
                        slab = spool.tile([128, slab_blk, H], F32, tag="slab")
                        goff = hf * tok_half + toff
                        g = nc.gpsimd.dma_gather(
                            slab[:, :ntok // 128, :], tab_ap,
                            idx_sb[:, goff // 16:(goff + ntok) // 16],
                            ntok, ntok, H, single_packet=False)
                        dep_lib(g)
                        for (sblk, ablk, nb) in pieces:
                            nc.vector.tensor_tensor(
                                acc_t[:, ablk:ablk + nb, :], acc_t[:, ablk:ablk + nb, :],
                                slab[:, sblk:sblk + nb, :], op=ALU.add)

                nc.sync.dma_start(accB_dram[:], accB[:])
                pslab = hpool.tile([128, nblk, H], F32, tag="pslab")
                done = 0
                while done < P:
                    n = min(MAX_GATHER, P - done)
                    g = nc.gpsimd.dma_gather(
                        pslab[:, done // 128:(done + n) // 128, :], accB_dram[:],
                        perm_sb[:, done // 16:(done + n) // 16], n, n, H,
                        single_packet=False)
                    dep_lib(g)
                    done += n

                for j in range(nblk):
                    v = wpool.tile([128, H], F32, tag="v")
                    nc.vector.tensor_tensor(v[:], acc[:, j, :], pslab[:, j, :], op=ALU.add)
                    nc.vector.tensor_scalar_mul(v[:], v[:], dinvs[:, j:j + 1])
                    nc.vector.tensor_tensor(v[:], v[:], bvs[:, layer * H:(layer + 1) * H],
                                            op=ALU.add)
                    sq = wpool.tile([128, H], F32, tag="sq")
                    ss = wpool.tile([128, 1], F32, tag="ss")
                    nc.scalar.activation(sq[:], v[:], AF.Square, accum_out=ss[:])
                    nrm = wpool.tile([128, 1], F32, tag="nrm")
                    nc.scalar.activation(nrm[:], ss[:], AF.Sqrt, bias=eps_t[:])
                    rinv = wpool.tile([128, 1], F32, tag="rinv")
                    nc.vector.reciprocal(rinv[:], nrm[:])
                    nc.scalar.activation(h[:, j, :], v[:], AF.Relu, scale=rinv[:])

            nemb_v = node_emb_o[:].rearrange("(b p) e -> b p e", p=128)
            for j in range(nblk):
                nc.sync.dma_start(nemb_v[j], h[:, j, :])

            gtiles = G // 128
            gp = []
            for t in range(gtiles):
                ps = papool.tile([128, H], F32, tag=f"poolps{t}", name=f"poolps{t}")
                gp.append(ps)
            for j in range(nblk):
                for t in range(gtiles):
                    nc.tensor.matmul(gp[t][:], Pps[:, j, t * 128:(t + 1) * 128],
                                     h[:, j, :], start=(j == 0), stop=(j == nblk - 1))
            pool_in_v = pool_in[:].rearrange("(t p) e -> t p e", p=128)
            for t in range(gtiles):
                sb = wpool.tile([128, H], F32, tag="poolsb")
                nc.vector.tensor_copy(sb[:], gp[t][:])
                nc.sync.dma_start(pool_in_v[t], sb[:])
            nc.gpsimd.collective_compute(
                "AllReduce", ALU.add, replica_groups=[list(range(C))],
                ins=[pool_in.opt()], outs=[pool_out.opt()])

            pool_out_v = pool_out[:].rearrange("(t p) e -> t p e", p=128)
            lo_v = logits_o[:].rearrange("(t p) e -> t p e", p=128)
            pr_v = probs_o[:].rearrange("(t p) e -> t p e", p=128)
            ge_v = gemb_o[:].rearrange("(t p) e -> t p e", p=128)
            for t in range(gtiles):
                emb = wpool.tile([128, H], F32, tag="emb")
                nc.sync.dma_start(emb[:], pool_out_v[t])
                psT = ppool.tile([H, 128], F32, tag="psT")
                nc.tensor.transpose(psT[:], emb[:], Is[:])
                embT = wpool.tile([H, 128], F32, tag="embTs")
                nc.scalar.copy(embT[:], psT[:])
                psl = ppool.tile([128, N_CLASSES], F32, tag="psm")
                nc.tensor.matmul(psl[:], embT[:], Wms[:], start=True, stop=True)
                lg = wpool.tile([128, N_CLASSES], F32, tag="lg")
                nc.vector.tensor_tensor(lg[:], psl[:], bms[:], op=ALU.add)
                mx = wpool.tile([128, 1], F32, tag="mx")
                nc.vector.tensor_reduce(mx[:], lg[:], axis=AX.X, op=ALU.max)
                mxn = wpool.tile([128, 1], F32, tag="mxn")
                nc.vector.tensor_scalar_mul(mxn[:], mx[:], -1.0)
                ex = wpool.tile([128, N_CLASSES], F32, tag="ex")
                ssum = wpool.tile([128, 1], F32, tag="ssum")
                nc.scalar.activation(ex[:], lg[:], AF.Exp, bias=mxn[:], accum_out=ssum[:])
                rs = wpool.tile([128, 1], F32, tag="rs")
                nc.vector.reciprocal(rs[:], ssum[:])
                pb = wpool.tile([128, N_CLASSES], F32, tag="pb")
                nc.vector.tensor_scalar_mul(pb[:], ex[:], rs[:])
                nc.sync.dma_start(lo_v[t], lg[:])
                nc.sync.dma_start(pr_v[t], pb[:])
                nc.sync.dma_start(ge_v[t], emb[:])

    nc.compile()
    return nc


_CACHE = {}


def kernel(x, edge_index, batch, W1, b1, W2, b2, W3, b3, Wm, bm):
    x = np.asarray(x, np.float32)
    N, F_IN = x.shape
    H = int(np.asarray(W1).shape[1])
    NCLS = int(np.asarray(Wm).shape[1])
    G = 256
    batch = np.asarray(batch)
    edge_index = np.asarray(edge_index)

    key = (N, F_IN, H, NCLS, int(edge_index.sum()), int(batch.sum()))
    if key in _CACHE:
        prep, nc = _CACHE[key]
    else:
        prep = _Prep(edge_index, batch, N, G, N_CORES)
        nc = _build_kernel(prep, F_IN, H, NCLS)
        _CACHE[key] = (prep, nc)

    ins = prep.core_inputs(x, W1, b1, W2, b2, W3, b3, Wm, bm)
    res = run_bass_kernel_spmd(nc, ins, core_ids=list(range(N_CORES)))
    node_emb = prep.assemble_node_emb([r["node_emb"] for r in res.results])
    r0 = res.results[0]
    return (np.asarray(r0["logits"], np.float32),
            np.asarray(r0["probs"], np.float32),
            node_emb.astype(np.float32),
            np.asarray(r0["graph_emb"], np.float32))


# revision 5
# speedup vs baseline: 98.4792x; 98.4792x over previous
"""Self-contained Trainium2 (Bass) kernel for nn_GCNClassifier: 3-layer GCN +
l2norm/relu + mean-pool readout, sharded across 8 NeuronCores.

Strategy (per core, nodes sharded 6250/core, stored in degree-ranked order):
  - GCN symmetric norm factorized into per-node scales:
        out = Dinv * (A_hat (Dinv * (h @ W))) + b
    so messages are raw gathered rows of the "table" t~ = Dinv*(h@W).
  - Per layer: transform own shard on PE, AllGather t~ into a full table in
    local HBM, then aggregate with dense degree-ranked gather "rounds":
    round r fetches the r-th in-edge source row for every target (targets
    sorted by in-degree so each round is a dense prefix); a DVE add
    accumulates.  No scatter-add needed anywhere in the layers.
  - dma_gather indices are int16, so the 50176-row table is split in two
    halves (source cores 0-3 / 4-7); half-B accumulates in its own degree
    order and is realigned with one extra dense permutation gather.
  - Readout: per-graph mean via one-hot matmul (weights 1/count), AllReduce,
    tiny head matmul + softmax.
"""

import numpy as np
import concourse.bacc as bacc
import concourse.mybir as mybir
import concourse.tile as tile
from concourse.tile_rust import add_dep_helper
from concourse.library_config import mlp as mlp_lib
from concourse.bass_utils import run_bass_kernel_spmd

F32 = mybir.dt.float32
I16 = mybir.dt.int16
AF = mybir.ActivationFunctionType
ALU = mybir.AluOpType
AX = mybir.AxisListType

MAX_GATHER = 3968          # idx/instruction; needs single_packet=False
N_CORES = 8


def _round_up(x, m):
    return (x + m - 1) // m * m


class _Prep:
    """Host-side graph preprocessing: indices, permutations, schedules."""

    def __init__(self, edge_index, batch, n_nodes, n_graphs, n_cores):
        N, C, G = n_nodes, n_cores, n_graphs
        assert N % C == 0
        NPC = N // C
        P = _round_up(NPC + 1, 128)   # >= 1 all-zero pad row for padding gathers
        self.N, self.C, self.G, self.NPC, self.P = N, C, G, NPC, P
        self.TAB = C * P
        self.HALF = (C // 2) * P

        ei = np.asarray(edge_index)
        row_all = np.concatenate([ei[0], np.arange(N, dtype=np.int64)])
        col_all = np.concatenate([ei[1], np.arange(N, dtype=np.int64)])
        deg = np.bincount(col_all, minlength=N).astype(np.float64)
        self.dinv = np.where(deg > 0, 1.0 / np.sqrt(deg), 0.0).astype(np.float32)
        # the appended self-loops are added on-chip as a direct shard add,
        # not gathered; original (i,i) edges stay in the token stream
        row, col = row_all[:len(ei[0])], col_all[:len(ei[0])]

        scor = row // NPC
        srcA = scor < (C // 2)
        degA = np.bincount(col[srcA], minlength=N)
        degB = np.bincount(col[~srcA], minlength=N)

        self.rankA_node = np.full((C, P), -1, np.int64)
        self.posA = np.empty(N, np.int64)
        self.posB = np.empty(N, np.int64)
        for k in range(C):
            g0 = k * NPC
            oa = np.argsort(-degA[g0:g0 + NPC], kind="stable")
            ob = np.argsort(-degB[g0:g0 + NPC], kind="stable")
            self.rankA_node[k, :NPC] = g0 + oa
            self.posA[g0 + oa] = np.arange(NPC)
            self.posB[g0 + ob] = np.arange(NPC)

        self.pid = (np.arange(N) // NPC) * P + self.posA

        tcore = col // NPC
        src_pid = self.pid[row]
        half = (src_pid >= self.HALF).astype(np.int64)
        src_idx = src_pid - half * self.HALF
        assert src_idx.max() < 32768, "int16 index overflow"

        tpos = np.where(half == 0, self.posA[col], self.posB[col])
        key = (tcore * 2 + half) * P + tpos
        order = np.argsort(key, kind="stable")
        e_s, e_c, e_h, e_t = src_idx[order], tcore[order], half[order], tpos[order]
        key_s = key[order]
        _, first_idx, counts = np.unique(key_s, return_index=True, return_counts=True)
        r_of = np.arange(len(key_s)) - np.repeat(first_idx, counts)

        degH = np.zeros((C, 2, P), np.int64)
        np.add.at(degH, (e_c, e_h, e_t), 1)
        maxdeg = int(degH.max())

        n_r = np.zeros((C, 2, maxdeg), np.int64)
        for k in range(C):
            for h in (0, 1):
                hist = np.bincount(degH[k, h], minlength=maxdeg + 1)
                ge = hist[::-1].cumsum()[::-1]
                n_r[k, h] = ge[1:maxdeg + 1]
        n_common = n_r.max(axis=(0, 1))
        n_pad = ((n_common + 127) // 128 * 128).astype(np.int64)

        offs = np.concatenate([[0], np.cumsum(n_pad)])
        self.rounds = [(int(offs[i]), int(n_pad[i])) for i in range(maxdeg)]
        self.tok_half = int(offs[-1])
        self.TOK = 2 * self.tok_half

        zrow = np.int16(NPC)   # a guaranteed all-zero table row, both halves
        tok = np.full((C, self.TOK), zrow, np.int16)
        flat_pos = e_h * self.tok_half + offs[r_of] + e_t
        tok[e_c, flat_pos] = e_s.astype(np.int16)
        self.tok = tok

        nblk = P // 128
        permB = np.zeros((C, P), np.int16)
        for k in range(C):
            nodes = self.rankA_node[k]
            real = nodes >= 0
            j = np.arange(P)
            j[real] = self.posB[nodes[real]]
            permB[k] = ((j % 128) * nblk + (j // 128)).astype(np.int16)
        self.permB = permB

        batch = np.asarray(batch)
        cnt = np.bincount(batch, minlength=G).astype(np.float32)
        self.inv_cnt = (1.0 / np.maximum(cnt, 1.0)).astype(np.float32)
        self.batch = batch

    def wrap16(self, arr_c):
        C, n = arr_c.shape
        a = arr_c.reshape(C, n // 16, 16).transpose(0, 2, 1)
        return np.tile(a, (1, 8, 1)).copy()

    def core_inputs(self, x, W1, b1, W2, b2, W3, b3, Wm, bm):
        C, P, G = self.C, self.P, self.G
        nblk = P // 128
        x = np.asarray(x, np.float32)
        H = np.asarray(W1).shape[1]
        tokw = self.wrap16(self.tok)
        permw = self.wrap16(self.permB)
        bvec = np.concatenate([np.asarray(b1), np.asarray(b2),
                               np.asarray(b3)]).astype(np.float32)
        b_rep = np.broadcast_to(bvec, (128, 3 * H)).copy()
        bm_arr = np.asarray(bm, np.float32)
        bm_rep = np.broadcast_to(bm_arr, (128, bm_arr.shape[0])).copy()
        ins = []
        for k in range(C):
            nodes = self.rankA_node[k]
            real = nodes >= 0
            xk = np.zeros((P, x.shape[1]), np.float32)
            xk[real] = x[nodes[real]]
            dv = np.zeros(P, np.float32)
            dv[real] = self.dinv[nodes[real]]
            pp = np.zeros((P, G), np.float32)
            pp[real, self.batch[nodes[real]]] = self.inv_cnt[self.batch[nodes[real]]]
            ins.append({
                "xT": np.ascontiguousarray(xk.T),
                "dinvq": np.ascontiguousarray(dv.reshape(nblk, 128).T),
                "idxs": tokw[k],
                "permB": permw[k],
                "Ppool": np.ascontiguousarray(pp.reshape(nblk, 128, G).transpose(1, 0, 2)),
                "W1": np.asarray(W1, np.float32), "W2": np.asarray(W2, np.float32),
                "W3": np.asarray(W3, np.float32), "Wm": np.asarray(Wm, np.float32),
                "bvec": b_rep, "bm_rep": bm_rep,
                "I128": np.eye(128, dtype=np.float32),
            })
        return ins

    def assemble_node_emb(self, shards):
        H = shards[0].shape[1]
        out = np.empty((self.N, H), np.float32)
        for k in range(self.C):
            nodes = self.rankA_node[k]
            real = nodes >= 0
            out[nodes[real]] = np.asarray(shards[k])[real]
        return out


def _make_chunks(rounds, slab_tok):
    """Split the per-half round list into gather chunks; pieces carry the
    slab-relative / acc-absolute block offsets for the accumulate adds."""
    chunks = []
    cur = None
    for (off, n) in rounds:
        done = 0
        while done < n:
            if cur is None:
                cur = [off + done, 0, []]
            take = min(n - done, slab_tok - cur[1])
            cur[2].append((cur[1] // 128, done // 128, take // 128))
            cur[1] += take
            done += take
            if cur[1] == slab_tok:
                chunks.append(tuple(cur))
                cur = None
    if cur is not None:
        chunks.append(tuple(cur))
    return chunks


def _build_kernel(prep, F_IN, H, N_CLASSES, slab_tok=MAX_GATHER):
    C, P, G, TAB, HALF = prep.C, prep.P, prep.G, prep.TAB, prep.HALF
    nblk = P // 128
    TOK, tok_half = prep.TOK, prep.tok_half
    slab_blk = slab_tok // 128
    chunks = _make_chunks(prep.rounds, slab_tok)
    assert G % 128 == 0

    nc = bacc.Bacc("TRN2", target_bir_lowering=False, debug=False, num_devices=C)
    xT = nc.dram_tensor("xT", [F_IN, P], F32, kind="ExternalInput")
    dinvq_d = nc.dram_tensor("dinvq", [128, nblk], F32, kind="ExternalInput")
    idxs_d = nc.dram_tensor("idxs", [128, TOK // 16], I16, kind="ExternalInput")
    permB_d = nc.dram_tensor("permB", [128, P // 16], I16, kind="ExternalInput")
    Ppool_d = nc.dram_tensor("Ppool", [128, nblk, G], F32, kind="ExternalInput")
    W1_d = nc.dram_tensor("W1", [F_IN, H], F32, kind="ExternalInput")
    W2_d = nc.dram_tensor("W2", [H, H], F32, kind="ExternalInput")
    W3_d = nc.dram_tensor("W3", [H, H], F32, kind="ExternalInput")
    Wm_d = nc.dram_tensor("Wm", [H, N_CLASSES], F32, kind="ExternalInput")
    bvec_d = nc.dram_tensor("bvec", [128, 3 * H], F32, kind="ExternalInput")
    bm_d = nc.dram_tensor("bm_rep", [128, N_CLASSES], F32, kind="ExternalInput")
    I_d = nc.dram_tensor("I128", [128, 128], F32, kind="ExternalInput")

    node_emb_o = nc.dram_tensor("node_emb", [P, H], F32, kind="ExternalOutput")
    logits_o = nc.dram_tensor("logits", [G, N_CLASSES], F32, kind="ExternalOutput")
    probs_o = nc.dram_tensor("probs", [G, N_CLASSES], F32, kind="ExternalOutput")
    gemb_o = nc.dram_tensor("graph_emb", [G, H], F32, kind="ExternalOutput")

    with tile.TileContext(nc) as tc:
        with (
            tc.tile_pool(name="const", bufs=1) as cpool,
            tc.tile_pool(name="state", bufs=1) as hpool,
            tc.tile_pool(name="slabs", bufs=3) as spool,
            tc.tile_pool(name="work", bufs=3) as wpool,
            tc.tile_pool(name="psum", bufs=2, space="PSUM") as ppool,
            tc.tile_pool(name="psacc", bufs=1, space="PSUM") as papool,
            tc.tile_pool(name="dram", bufs=1, space="DRAM") as dpool,
        ):
            lib = nc.gpsimd.load_library(mlp_lib)

            def dep_lib(ins_obj):
                add_dep_helper(ins_obj.ins, lib.ins, sync=False, reason="lib first")

            W1s = cpool.tile([F_IN, H], F32); nc.sync.dma_start(W1s[:], W1_d[:])
            W2s = cpool.tile([H, H], F32);   nc.sync.dma_start(W2s[:], W2_d[:])
            W3s = cpool.tile([H, H], F32);   nc.sync.dma_start(W3s[:], W3_d[:])
            Wms = cpool.tile([H, N_CLASSES], F32); nc.sync.dma_start(Wms[:], Wm_d[:])
            bvs = cpool.tile([128, 3 * H], F32); nc.sync.dma_start(bvs[:], bvec_d[:])
            bms = cpool.tile([128, N_CLASSES], F32); nc.sync.dma_start(bms[:], bm_d[:])
            Is = cpool.tile([128, 128], F32); nc.sync.dma_start(Is[:], I_d[:])
            dinvs = cpool.tile([128, nblk], F32); nc.sync.dma_start(dinvs[:], dinvq_d[:])
            eps_t = cpool.tile([128, 1], F32); nc.vector.memset(eps_t[:], 1e-24)
            idx_sb = cpool.tile([128, TOK // 16], I16); nc.sync.dma_start(idx_sb[:], idxs_d[:])
            perm_sb = cpool.tile([128, P // 16], I16); nc.sync.dma_start(perm_sb[:], permB_d[:])
            Pps = cpool.tile([128, nblk, G], F32); nc.sync.dma_start(Pps[:], Ppool_d[:])

            h = hpool.tile([128, nblk, H], F32, tag="h")
            t_own = hpool.tile([128, nblk, H], F32, tag="town")
            acc = hpool.tile([128, nblk, H], F32, tag="accA")
            accB = hpool.tile([128, nblk, H], F32, tag="accB")

            tshard = dpool.tile([P, H], F32)
            tables = [dpool.tile([TAB, H], F32, addr_space="Shared", name=f"table{i}")
                      for i in range(3)]
            accB_dram = dpool.tile([128 * nblk, H], F32)
            pool_in = dpool.tile([G, H], F32)
            pool_out = dpool.tile([G, H], F32, addr_space="Shared")

            tshard_v = tshard[:].rearrange("(b p) e -> b p e", p=128)

            Ws = [W1s, W2s, W3s]
            for layer in range(3):
                for j in range(nblk):
                    if layer == 0:
                        lhsT = wpool.tile([F_IN, 128], F32, tag="xt")
                        nc.sync.dma_start(lhsT[:], xT[:, j * 128:(j + 1) * 128])
                    else:
                        pst = ppool.tile([H, 128], F32, tag="psT")
                        nc.tensor.transpose(pst[:], h[:, j, :], Is[:])
                        lhsT = wpool.tile([H, 128], F32, tag="hT")
                        nc.scalar.copy(lhsT[:], pst[:])
                    psm = ppool.tile([128, H], F32, tag="psm")
                    nc.tensor.matmul(psm[:], lhsT[:], Ws[layer][:], start=True, stop=True)
                    nc.vector.tensor_scalar_mul(t_own[:, j, :], psm[:], dinvs[:, j:j + 1])
                    nc.sync.dma_start(tshard_v[j], t_own[:, j, :])

                table = tables[layer]
                nc.gpsimd.collective_compute(
                    "AllGather", ALU.bypass, replica_groups=[list(range(C))],
                    ins=[tshard.opt()], outs=[table.opt()])

                # self-loop contribution: acc starts as the own shard t~
                nc.vector.tensor_copy(acc[:], t_own[:])
                nc.vector.memset(accB[:], 0.0)
                for hf in (0, 1):
                    acc_t = acc if hf == 0 else accB
                    tab_ap = table[hf * HALF:(hf + 1) * HALF, :]
                    for (toff, ntok, pieces) in chunks:
                        slab = spool.tile([128, slab_blk, H], F32, tag="slab")
                        goff = hf * tok_half + toff
                        g = nc.gpsimd.dma_gather(
                            slab[:, :ntok // 128, :], tab_ap,
                            idx_sb[:, goff // 16:(goff + ntok) // 16],
                            ntok, ntok, H, single_packet=False)
                        dep_lib(g)
                        for (sblk, ablk, nb) in pieces:
                            nc.vector.tensor_tensor(
                                acc_t[:, ablk:ablk + nb, :], acc_t[:, ablk:ablk + nb, :],
                                slab[:, sblk:sblk + nb, :], op=ALU.add)

                nc.sync.dma_start(accB_dram[:], accB[:])
                pslab = hpool.tile([128, nblk, H], F32, tag="pslab")
                done = 0
                while done < P:
                    n = min(MAX_GATHER, P - done)
                    g = nc.gpsimd.dma_gather(
                        pslab[:, done // 128:(done + n) // 128, :], accB_dram[:],
                        perm_sb[:, done // 16:(done + n) // 16], n, n, H,
                        single_packet=False)
                    dep_lib(g)
                    done += n

                for j in range(nblk):
                    v = wpool.tile([128, H], F32, tag="v")
                    nc.vector.tensor_tensor(v[:], acc[:, j, :], pslab[:, j, :], op=ALU.add)
                    nc.vector.tensor_scalar_mul(v[:], v[:], dinvs[:, j:j + 1])
                    nc.vector.tensor_tensor(v[:], v[:], bvs[:, layer * H:(layer + 1) * H],
                                            op=ALU.add)
                    sq = wpool.tile([128, H], F32, tag="sq")
                    ss = wpool.tile([128, 1], F32, tag="ss")
                    nc.scalar.activation(sq[:], v[:], AF.Square, accum_out=ss[:])
                    nrm = wpool.tile([128, 1], F32, tag="nrm")
                    nc.scalar.activation(nrm[:], ss[:], AF.Sqrt, bias=eps_t[:])
                    rinv = wpool.tile([128, 1], F32, tag="rinv")
                    nc.vector.reciprocal(rinv[:], nrm[:])
                    nc.scalar.activation(h[:, j, :], v[:], AF.Relu, scale=rinv[:])

            nemb_v = node_emb_o[:].rearrange("(b p) e -> b p e", p=128)
            for j in range(nblk):
                nc.sync.dma_start(nemb_v[j], h[:, j, :])

            gtiles = G // 128
            gp = []
            for t in range(gtiles):
                ps = papool.tile([128, H], F32, tag=f"poolps{t}", name=f"poolps{t}")
                gp.append(ps)
            for j in range(nblk):
                for t in range(gtiles):
                    nc.tensor.matmul(gp[t][:], Pps[:, j, t * 128:(t + 1) * 128],
                                     h[:, j, :], start=(j == 0), stop=(j == nblk - 1))
            pool_in_v = pool_in[:].rearrange("(t p) e -> t p e", p=128)
            for t in range(gtiles):
                sb = wpool.tile([128, H], F32, tag="poolsb")
                nc.vector.tensor_copy(sb[:], gp[t][:])
                nc.sync.dma_start(pool_in_v[t], sb[:])
            nc.gpsimd.collective_compute(
                "AllReduce", ALU.add, replica_groups=[list(range(C))],
                ins=[pool_in.opt()], outs=[pool_out.opt()])

            pool_out_v = pool_out[:].rearrange("(t p) e -> t p e", p=128)
            lo_v = logits_o[:].rearrange("(t p) e -> t p e", p=128)
            pr_v = probs_o[:].rearrange("(t p) e -> t p e", p=128)
            ge_v = gemb_o[:].rearrange("(t p) e -> t p e", p=128)
            for t in range(gtiles):
                emb = wpool.tile([128, H], F32, tag="emb")
                nc.sync.dma_start(emb[:], pool_out_v[t])
                psT = ppool.tile([H, 128], F32, tag="psT")
                nc.tensor.transpose(psT[:], emb[:], Is[:])
                embT = wpool.tile([H, 128], F32, tag="embTs")
                nc.scalar.copy(embT[:], psT[:])
                psl = ppool.tile([128, N_CLASSES], F32, tag="psm")
                nc.tensor.matmul(psl[:], embT[:], Wms[:], start=True, stop=True)
                lg = wpool.tile([128, N_CLASSES], F32, tag="lg")
                nc.vector.tensor_tensor(lg[:], psl[:], bms[:], op=ALU.add)
                mx = wpool.tile([128, 1], F32, tag="mx")
                nc.vector.tensor_reduce(mx[:], lg[:], axis=AX.X, op=ALU.max)
                mxn = wpool.tile([128, 1], F32, tag="mxn")
                nc.vector.tensor_scalar_mul(mxn[:], mx[:], -1.0)
                ex = wpool.tile([128, N_CLASSES], F32, tag="ex")
                ssum = wpool.tile([128, 1], F32, tag="ssum")
                nc.scalar.activation(ex[:], lg[:], AF.Exp, bias=mxn[:], accum_out=ssum[:])
                rs = wpool.tile([128, 1], F32, tag="rs")
                nc.vector.reciprocal(rs[:], ssum[:])
                pb = wpool.tile([128, N_CLASSES], F32, tag="pb")
                nc.vector.tensor_scalar_mul(pb[:], ex[:], rs[:])
                nc.sync.dma_start(lo_v[t], lg[:])
                nc.sync.dma_start(pr_v[t], pb[:])
                nc.sync.dma_start(ge_v[t], emb[:])

    nc.compile()
    return nc


_CACHE = {}


def kernel(x, edge_index, batch, W1, b1, W2, b2, W3, b3, Wm, bm):
    x = np.asarray(x, np.float32)
    N, F_IN = x.shape
    H = int(np.asarray(W1).shape[1])
    NCLS = int(np.asarray(Wm).shape[1])
    G = 256
    batch = np.asarray(batch)
    edge_index = np.asarray(edge_index)

    key = (N, F_IN, H, NCLS, int(edge_index.sum()), int(batch.sum()))
    if key in _CACHE:
        prep, nc = _CACHE[key]
    else:
        prep = _Prep(edge_index, batch, N, G, N_CORES)
        nc = _build_kernel(prep, F_IN, H, NCLS)
        _CACHE[key] = (prep, nc)

    ins = prep.core_inputs(x, W1, b1, W2, b2, W3, b3, Wm, bm)
    res = run_bass_kernel_spmd(nc, ins, core_ids=list(range(N_CORES)))
    node_emb = prep.assemble_node_emb([r["node_emb"] for r in res.results])
    r0 = res.results[0]
    return (np.asarray(r0["logits"], np.float32),
            np.asarray(r0["probs"], np.float32),
            node_emb.astype(np.float32),
            np.asarray(r0["graph_emb"], np.float32))
